# revision 29
# baseline (speedup 1.0000x reference)
"""Trainium2 Bass kernel for nn_AbstractLinear (IBP linear layer).

Computes, for x (256, 8192), W (8192, 8192), b, low, high (8192,):
    y        = x @ W.T + b
    low_out  = W+ @ low + W- @ high + b
    high_out = W+ @ high + W- @ low + b

Sharding: 2D grid over the 8 cores — W split O_SHARDS-wise along output rows
and I_SHARDS-wise along the contraction dim. Each core handles a
(OPC x IN_PC) block of W; host sums the I_SHARDS partials per output block
and concatenates (no on-device collectives).

Math used on-device (per core, o = its OPC output rows, i = its IN_PC slice):
    out_y[o, 0:256] = sum_i wt[i,o] * x.T[i, :]      (partial y.T block)
    out_y[o, 256]   = partial (W @ low)[o]           (extra moving column)
    out_y[o, 257]   = partial (W @ high)[o]          (extra moving column)
    out_s[o]        = partial sum_i relu(W[o,i]) * (high-low)[i]
Host assembly (after summing i-shard partials):
    y        = out_y[:, :256].T + b
    low_out  = out_y[:, 257] - out_s + b
    high_out = out_y[:, 256] + out_s + b

W and x travel in fp16 (same 11-bit-mantissa class as the PE's fp32r/TF32
matmul path, but half the HBM bytes and FWL-accelerated weight loads);
measured rel err ~2e-4 on y, ~4e-5 on the bounds. The relu matvec is
computed as elementwise relu(wt * (high-low)) on ACT/DVE (alternating
i-tiles), 4 tiles tree-summed on DVE in fp16, then reduced over partitions
with a ones-vector fp16 matmul accumulating in PSUM.
"""

import numpy as np

import concourse.bass as bass
import concourse.tile as tile
from concourse import bacc, mybir
from concourse.bass_utils import run_bass_kernel_spmd

FP32R = mybir.dt.float32r
FP32 = mybir.dt.float32
FP16 = mybir.dt.float16
BF16 = mybir.dt.bfloat16

# W and x travel in fp16: same 11-bit mantissa class as the fp32r (TF32)
# matmul path, but half the HBM bytes, FWL-accelerated weight loads and
# 1 cycle/row unconditionally.
WDT = FP16
WNP = __import__("numpy").float16

IN = 8192  # total contraction dim i
OUT = 8192  # total output features
B = 256  # batch
NCORES = 8
O_SHARDS = 8
I_SHARDS = NCORES // O_SHARDS
OPC = OUT // O_SHARDS  # output features per core
IN_PC = IN // I_SHARDS  # contraction per core
NB = B + 2  # moving columns: 256 batch + low + high
IT = IN_PC // 128  # i-tiles per core
HALF = 512  # o columns processed per phase
NPH = OPC // HALF  # phases
OSUB = HALF // 128  # 4 psum tiles per phase
BLKI = 8  # i-tiles per steady-state wt DMA (1 MiB fp16 transfers)


def _block_schedule():
    # Ladder up: small leading transfers so the PE starts ~10 us sooner, then
    # 2 MiB steady-state blocks, then taper down so the final relu/add/s-chain
    # isn't bunched behind one big block at the end. Sizes stay multiples of 4
    # (after the first two) so s-quads never straddle a block.
    head = [2, 2, 4, 8]
    tail = [8, 4, 4]
    blocks = []
    t = 0
    for n in head:
        if t + n <= IT - sum(tail):
            blocks.append((t, n))
            t += n
    while t < IT - sum(tail):
        n = min(BLKI, IT - sum(tail) - t)
        blocks.append((t, n))
        t += n
    for n in tail:
        if t + n <= IT:
            blocks.append((t, n))
            t += n
    while t < IT:
        blocks.append((t, min(4, IT - t)))
        t += min(4, IT - t)
    return blocks


BLOCKS = _block_schedule()

_NC_CACHE = {}


def _build():
    nc = bacc.Bacc("TRN2", target_bir_lowering=False, debug=False, num_devices=NCORES)
    # wt is host-pre-tiled to [phase, partition, i-tile, o-col] so every DMA
    # reads per-partition-contiguous 2 KiB * nt runs (16 KiB descriptors at
    # steady state) instead of scattered 2 KiB chunks.
    wt_d = nc.dram_tensor("wt", [NPH, 128, IT, HALF], WDT, kind="ExternalInput")
    # xlh likewise pre-tiled to [partition, i-tile, col].
    xlh_d = nc.dram_tensor("xlh", [128, IT, NB], WDT, kind="ExternalInput")
    absu_d = nc.dram_tensor("absu", [128, IT], FP32, kind="ExternalInput")
    outy_d = nc.dram_tensor("out_y", [OPC, NB], FP32, kind="ExternalOutput")
    outs_d = nc.dram_tensor("out_s", [1, OPC], FP32, kind="ExternalOutput")

    with tile.TileContext(nc) as tc:
        with (
            tc.tile_pool(name="const", bufs=1) as constp,
            tc.tile_pool(name="data", bufs=1) as datap,
            tc.tile_pool(name="wtp", bufs=10) as wtp,
            tc.tile_pool(name="tmpp", bufs=3) as tmpp,
            tc.tile_pool(name="tsump", bufs=5) as tsump,
            tc.tile_pool(name="outp", bufs=3) as outp,
            # py0..py2 double-buffered so the next phase's matmuls start while
            # the previous phase's psum drains; py3+ps single (8 banks total).
            tc.tile_pool(name="psumA", bufs=2, space="PSUM") as psumpa,
            tc.tile_pool(name="psumB", bufs=1, space="PSUM") as psumpb,
        ):
            ones = constp.tile([128, 1], FP16)
            nc.vector.memset(ones[:], 1.0)
            zbias = constp.tile([128, 1], FP32)
            nc.vector.memset(zbias[:], 0.0)

            # xlh/absu/outputs go through the ACT HWDGE ring; wt tiles through
            # the SP ring — otherwise the xlh load FIFO-blocks the first
            # weight tile and the PE idles for its duration.
            XLH = datap.tile([128, IT, NB], WDT)
            AU = datap.tile([128, IT], FP32)
            for k, (t0, nt) in enumerate(BLOCKS):
                xeng = nc.gpsimd if k < 2 else nc.scalar
                xeng.dma_start(
                    out=XLH[:, t0 : t0 + nt, :],
                    in_=xlh_d[:, t0 : t0 + nt, :],
                )
                if k == 0:
                    nc.scalar.dma_start(out=AU[:], in_=absu_d[:])

            ssb = outp.tile([1, OPC], FP32, tag="ssb")

            for h in range(NPH):
                psum_y = [
                    (psumpa if j < 3 else psumpb).tile(
                        [128, NB], FP32, name=f"py{h}_{j}", tag=f"py{j}"
                    )
                    for j in range(OSUB)
                ]
                psum_s = psumpb.tile([1, HALF], FP32, name=f"ps{h}", tag="ps")
                pend_s = []
                first_s = [True]

                w4_of = {}
                for kb, (t0, nt) in enumerate(BLOCKS):
                    w4 = wtp.tile([128, nt, HALF], WDT, tag="w4")
                    weng = nc.gpsimd if (h == 0 and kb < 2) else nc.sync
                    weng.dma_start(
                        out=w4[:],
                        in_=wt_d[h, :, t0 : t0 + nt, :],
                    )
                    for j in range(nt):
                        t = t0 + j
                        w4_of[t] = (w4, j)
                        if t == 2:
                            # Catch up the deferred osub3 matmuls (t=0 has
                            # start=True, so it must precede t=2's osub3).
                            for td in (0, 1):
                                wb, jb = w4_of[td]
                                nc.tensor.matmul(
                                    psum_y[OSUB - 1][:],
                                    wb[:, jb, 128 * (OSUB - 1) : 128 * OSUB],
                                    XLH[:, td, :],
                                    start=(td == 0),
                                    stop=False,
                                )
                        for osub in range(OSUB):
                            # Defer the first two i-tiles' osub3 matmuls: py3
                            # is single-buffered, so give its drain from the
                            # previous phase ~1 us of PE cover.
                            if t < 2 and osub == OSUB - 1:
                                continue
                            nc.tensor.matmul(
                                psum_y[osub][:],
                                w4[:, j, 128 * osub : 128 * (osub + 1)],
                                XLH[:, t, :],
                                start=(t == 0) and osub != OSUB - 1,
                                stop=(t == IT - 1),
                            )
                        # relu(wt * (high-low)): even i-tiles on ACT, odd on
                        # DVE; 4 tiles tree-summed on DVE so only one
                        # PSUM-reducing matmul per four i-tiles.
                        if j % 2 == 0:
                            tmpe = tmpp.tile([128, HALF], FP16, tag="tmpe")
                            nc.scalar.activation(
                                tmpe[:],
                                w4[:, j, :],
                                mybir.ActivationFunctionType.Relu,
                                bias=zbias[:],
                                scale=AU[:, t : t + 1],
                            )
                        else:
                            tmpo = tmpp.tile([128, HALF], FP16, tag="tmpo")
                            nc.vector.tensor_scalar(
                                tmpo[:],
                                w4[:, j, :],
                                0.0,
                                AU[:, t : t + 1],
                                mybir.AluOpType.max,
                                mybir.AluOpType.mult,
                            )
                            if t % 4 == 1:
                                tsumA = tmpp.tile([128, HALF], FP16, tag="tsumA")
                                nc.vector.tensor_add(tsumA[:], tmpe[:], tmpo[:])
                            else:
                                tsumB = tmpp.tile([128, HALF], FP16, tag="tsumB")
                                nc.vector.tensor_add(tsumB[:], tmpe[:], tmpo[:])
                                tsum = tsump.tile([128, HALF], FP16, tag="tsum")
                                nc.vector.tensor_add(tsum[:], tsumA[:], tsumB[:])
                                # psum_s is single-buffered and its tsum
                                # operand comes off the (laggy) DVE chain:
                                # keep a rolling 2-quad deferral so the PE
                                # never waits on either, tapering to 0 at the
                                # end of the phase.
                                pend_s.append(tsum)
                                keep = 2 if t < IT - 9 else (1 if t < IT - 1 else 0)
                                while len(pend_s) > keep:
                                    ts2 = pend_s.pop(0)
                                    nc.tensor.matmul(
                                        psum_s[:],
                                        ones[:],
                                        ts2[:],
                                        start=first_s[0],
                                        stop=(t == IT - 1 and not pend_s),
                                    )
                                    first_s[0] = False

                # drain py3 first: the next phase's deferred osub3 matmuls
                # are what ultimately wait on this bank.
                for osub in (3, 0, 1, 2):
                    ysb = outp.tile([128, NB], FP32, tag="ysb")
                    nc.vector.tensor_copy(ysb[:], psum_y[osub][:])
                    o0 = HALF * h + 128 * osub
                    nc.scalar.dma_start(out=outy_d[o0 : o0 + 128, :], in_=ysb[:])
                nc.vector.tensor_copy(
                    ssb[:, HALF * h : HALF * (h + 1)], psum_s[:]
                )
            nc.scalar.dma_start(out=outs_d[:], in_=ssb[:])

    nc.compile()
    return nc


def get_nc():
    if "nc" not in _NC_CACHE:
        _NC_CACHE["nc"] = _build()
    return _NC_CACHE["nc"]


def make_in_maps(x, low, high, W):
    xt = x.T  # (IN, B) view
    au = (high - low).astype(np.float32)
    in_maps = []
    for c in range(NCORES):
        q, r = divmod(c, I_SHARDS)  # q: o-shard, r: i-shard
        isl = slice(IN_PC * r, IN_PC * (r + 1))
        xlh_flat = np.concatenate(
            [xt[isl], low[isl, None], high[isl, None]], axis=1
        ).astype(np.float32)  # (IN_PC, NB)
        # -> [partition, i-tile, col]
        xlh = np.ascontiguousarray(
            xlh_flat.reshape(IT, 128, NB).transpose(1, 0, 2).astype(WNP)
        )
        absu = np.ascontiguousarray(au[isl].reshape(IT, 128).T)
        # W block (OPC, IN_PC) -> [phase, partition, i-tile, o-col]:
        # wt[h, p, t, f] = W[OPC*q + HALF*h + f, IN_PC*r + 128*t + p]
        wblk = W[OPC * q : OPC * (q + 1), isl]
        wt = np.ascontiguousarray(
            wblk.reshape(NPH, HALF, IT, 128).transpose(0, 3, 2, 1).astype(WNP)
        )
        in_maps.append({"wt": wt, "xlh": xlh, "absu": absu})
    return in_maps


def assemble(results, b):
    yblocks = []
    sblocks = []
    for q in range(O_SHARDS):
        yb = results[q * I_SHARDS]["out_y"]
        sb = results[q * I_SHARDS]["out_s"][0]
        for r in range(1, I_SHARDS):
            yb = yb + results[q * I_SHARDS + r]["out_y"]
            sb = sb + results[q * I_SHARDS + r]["out_s"][0]
        yblocks.append(yb)
        sblocks.append(sb)
    ycols = np.concatenate(yblocks, axis=0)  # (8192, 258)
    s = np.concatenate(sblocks)  # (8192,)
    y = ycols[:, :B].T + b[None, :]
    t_low = ycols[:, B]
    t_high = ycols[:, B + 1]
    low_out = t_high - s + b
    high_out = t_low + s + b
    return (
        np.ascontiguousarray(y, dtype=np.float32),
        low_out.astype(np.float32),
        high_out.astype(np.float32),
    )


def kernel(x, low, high, W, b):
    x = np.asarray(x, dtype=np.float32)
    low = np.asarray(low, dtype=np.float32)
    high = np.asarray(high, dtype=np.float32)
    W = np.asarray(W, dtype=np.float32)
    b = np.asarray(b, dtype=np.float32)

    nc = get_nc()
    in_maps = make_in_maps(x, low, high, W)
    r = run_bass_kernel_spmd(nc, in_maps, list(range(NCORES)))
    return assemble(r.results, b)


# revision 30
# speedup vs baseline: 1.1686x; 1.1686x over previous
"""Trainium2 Bass kernel for nn_AbstractLinear (IBP linear layer).

Computes, for x (256, 8192), W (8192, 8192), b, low, high (8192,):
    y        = x @ W.T + b
    low_out  = W+ @ low + W- @ high + b
    high_out = W+ @ high + W- @ low + b

Sharding: 2D grid over the 8 cores — W split O_SHARDS-wise along output rows
and I_SHARDS-wise along the contraction dim. Each core handles a
(OPC x IN_PC) block of W; host sums the I_SHARDS partials per output block
and concatenates (no on-device collectives).

Math used on-device (per core, o = its OPC output rows, i = its IN_PC slice):
    out_y[o, 0:256] = sum_i wt[i,o] * x.T[i, :]      (partial y.T block)
    out_y[o, 256]   = partial (W @ low)[o]           (extra moving column)
    out_y[o, 257]   = partial (W @ high)[o]          (extra moving column)
    out_s[o]        = partial sum_i relu(W[o,i]) * (high-low)[i]
Host assembly (after summing i-shard partials):
    y        = out_y[:, :256].T + b
    low_out  = out_y[:, 257] - out_s + b
    high_out = out_y[:, 256] + out_s + b

W and x travel in fp16 (same 11-bit-mantissa class as the PE's fp32r/TF32
matmul path, but half the HBM bytes and FWL-accelerated weight loads);
measured rel err ~2e-4 on y, ~4e-5 on the bounds. The relu matvec is
computed as elementwise relu(wt * (high-low)) on ACT/DVE (alternating
i-tiles), 4 tiles tree-summed on DVE in fp16, then reduced over partitions
with a ones-vector fp16 matmul accumulating in PSUM.
"""

import numpy as np

import concourse.bass as bass
import concourse.tile as tile
from concourse import bacc, mybir
from concourse.bass_utils import run_bass_kernel_spmd

FP32R = mybir.dt.float32r
FP32 = mybir.dt.float32
FP16 = mybir.dt.float16
BF16 = mybir.dt.bfloat16

# W and x travel in fp16: same 11-bit mantissa class as the fp32r (TF32)
# matmul path, but half the HBM bytes, FWL-accelerated weight loads and
# 1 cycle/row unconditionally.
WDT = FP16
WNP = __import__("numpy").float16

IN = 8192  # total contraction dim i
OUT = 8192  # total output features
B = 256  # batch
NCORES = 8
O_SHARDS = 8
I_SHARDS = NCORES // O_SHARDS
OPC = OUT // O_SHARDS  # output features per core
IN_PC = IN // I_SHARDS  # contraction per core
NB = B + 2  # moving columns: 256 batch + low + high
IT = IN_PC // 128  # i-tiles per core
HALF = 512  # o columns processed per phase
NPH = OPC // HALF  # phases
OSUB = HALF // 128  # 4 psum tiles per phase
BLKI = 8  # i-tiles per steady-state wt DMA (1 MiB fp16 transfers)


def _block_schedule():
    # Ladder up: small leading transfers so the PE starts ~10 us sooner, then
    # 2 MiB steady-state blocks, then taper down so the final relu/add/s-chain
    # isn't bunched behind one big block at the end. Sizes stay multiples of 4
    # (after the first two) so s-quads never straddle a block.
    head = [2, 2, 4, 8]
    tail = [8, 4, 4]
    blocks = []
    t = 0
    for n in head:
        if t + n <= IT - sum(tail):
            blocks.append((t, n))
            t += n
    while t < IT - sum(tail):
        n = min(BLKI, IT - sum(tail) - t)
        blocks.append((t, n))
        t += n
    for n in tail:
        if t + n <= IT:
            blocks.append((t, n))
            t += n
    while t < IT:
        blocks.append((t, min(4, IT - t)))
        t += min(4, IT - t)
    return blocks


BLOCKS = _block_schedule()

_NC_CACHE = {}


def _build():
    nc = bacc.Bacc("TRN2", target_bir_lowering=False, debug=False, num_devices=NCORES)
    # wt is host-pre-tiled to [phase, partition, i-tile, o-col] so every DMA
    # reads per-partition-contiguous 2 KiB * nt runs (16 KiB descriptors at
    # steady state) instead of scattered 2 KiB chunks.
    wt_d = nc.dram_tensor("wt", [NPH, 128, IT, HALF], WDT, kind="ExternalInput")
    # xlh likewise pre-tiled to [partition, i-tile, col].
    xlh_d = nc.dram_tensor("xlh", [128, IT, NB], WDT, kind="ExternalInput")
    absu_d = nc.dram_tensor("absu", [128, IT], FP32, kind="ExternalInput")
    outy_d = nc.dram_tensor("out_y", [OPC, NB], FP32, kind="ExternalOutput")
    outs_d = nc.dram_tensor("out_s", [1, OPC], FP32, kind="ExternalOutput")

    with tile.TileContext(nc) as tc:
        with (
            tc.tile_pool(name="const", bufs=1) as constp,
            tc.tile_pool(name="data", bufs=1) as datap,
            tc.tile_pool(name="wtp", bufs=10) as wtp,
            tc.tile_pool(name="tmpp", bufs=3) as tmpp,
            tc.tile_pool(name="tsump", bufs=5) as tsump,
            tc.tile_pool(name="outp", bufs=3) as outp,
            # py0..py2 double-buffered so the next phase's matmuls start while
            # the previous phase's psum drains; py3+ps single (8 banks total).
            tc.tile_pool(name="psumA", bufs=2, space="PSUM") as psumpa,
            tc.tile_pool(name="psumB", bufs=1, space="PSUM") as psumpb,
        ):
            ones = constp.tile([128, 1], FP16)
            nc.vector.memset(ones[:], 1.0)
            zbias = constp.tile([128, 1], FP32)
            nc.vector.memset(zbias[:], 0.0)

            # xlh/absu/outputs go through the ACT HWDGE ring; wt tiles through
            # the SP ring — otherwise the xlh load FIFO-blocks the first
            # weight tile and the PE idles for its duration.
            XLH = datap.tile([128, IT, NB], WDT)
            AU = datap.tile([128, IT], FP32)
            for k, (t0, nt) in enumerate(BLOCKS):
                nc.scalar.dma_start(
                    out=XLH[:, t0 : t0 + nt, :],
                    in_=xlh_d[:, t0 : t0 + nt, :],
                )
                if k == 0:
                    nc.scalar.dma_start(out=AU[:], in_=absu_d[:])

            ssb = outp.tile([1, OPC], FP32, tag="ssb")

            for h in range(NPH):
                psum_y = [
                    (psumpa if j < 3 else psumpb).tile(
                        [128, NB], FP32, name=f"py{h}_{j}", tag=f"py{j}"
                    )
                    for j in range(OSUB)
                ]
                psum_s = psumpb.tile([1, HALF], FP32, name=f"ps{h}", tag="ps")
                pend_s = []
                first_s = [True]

                w4_of = {}
                for kb, (t0, nt) in enumerate(BLOCKS):
                    w4 = wtp.tile([128, nt, HALF], WDT, tag="w4")
                    nc.sync.dma_start(
                        out=w4[:],
                        in_=wt_d[h, :, t0 : t0 + nt, :],
                    )
                    for j in range(nt):
                        t = t0 + j
                        w4_of[t] = (w4, j)
                        if t == 2:
                            # Catch up the deferred osub3 matmuls (t=0 has
                            # start=True, so it must precede t=2's osub3).
                            for td in (0, 1):
                                wb, jb = w4_of[td]
                                nc.tensor.matmul(
                                    psum_y[OSUB - 1][:],
                                    wb[:, jb, 128 * (OSUB - 1) : 128 * OSUB],
                                    XLH[:, td, :],
                                    start=(td == 0),
                                    stop=False,
                                )
                        for osub in range(OSUB):
                            # Defer the first two i-tiles' osub3 matmuls: py3
                            # is single-buffered, so give its drain from the
                            # previous phase ~1 us of PE cover.
                            if t < 2 and osub == OSUB - 1:
                                continue
                            nc.tensor.matmul(
                                psum_y[osub][:],
                                w4[:, j, 128 * osub : 128 * (osub + 1)],
                                XLH[:, t, :],
                                start=(t == 0) and osub != OSUB - 1,
                                stop=(t == IT - 1),
                            )
                        # relu(wt * (high-low)): even i-tiles on ACT, odd on
                        # DVE; 4 tiles tree-summed on DVE so only one
                        # PSUM-reducing matmul per four i-tiles.
                        if j % 2 == 0:
                            tmpe = tmpp.tile([128, HALF], FP16, tag="tmpe")
                            nc.scalar.activation(
                                tmpe[:],
                                w4[:, j, :],
                                mybir.ActivationFunctionType.Relu,
                                bias=zbias[:],
                                scale=AU[:, t : t + 1],
                            )
                        else:
                            tmpo = tmpp.tile([128, HALF], FP16, tag="tmpo")
                            nc.vector.tensor_scalar(
                                tmpo[:],
                                w4[:, j, :],
                                0.0,
                                AU[:, t : t + 1],
                                mybir.AluOpType.max,
                                mybir.AluOpType.mult,
                            )
                            if t % 4 == 1:
                                tsumA = tmpp.tile([128, HALF], FP16, tag="tsumA")
                                nc.vector.tensor_add(tsumA[:], tmpe[:], tmpo[:])
                            else:
                                tsumB = tmpp.tile([128, HALF], FP16, tag="tsumB")
                                nc.vector.tensor_add(tsumB[:], tmpe[:], tmpo[:])
                                tsum = tsump.tile([128, HALF], FP16, tag="tsum")
                                nc.vector.tensor_add(tsum[:], tsumA[:], tsumB[:])
                                # psum_s is single-buffered and its tsum
                                # operand comes off the (laggy) DVE chain:
                                # keep a rolling 2-quad deferral so the PE
                                # never waits on either, tapering to 0 at the
                                # end of the phase.
                                pend_s.append(tsum)
                                keep = 2 if t < IT - 9 else (1 if t < IT - 1 else 0)
                                while len(pend_s) > keep:
                                    ts2 = pend_s.pop(0)
                                    nc.tensor.matmul(
                                        psum_s[:],
                                        ones[:],
                                        ts2[:],
                                        start=first_s[0],
                                        stop=(t == IT - 1 and not pend_s),
                                    )
                                    first_s[0] = False

                # drain py3 first: the next phase's deferred osub3 matmuls
                # are what ultimately wait on this bank.
                for osub in (3, 0, 1, 2):
                    ysb = outp.tile([128, NB], FP32, tag="ysb")
                    nc.vector.tensor_copy(ysb[:], psum_y[osub][:])
                    o0 = HALF * h + 128 * osub
                    nc.scalar.dma_start(out=outy_d[o0 : o0 + 128, :], in_=ysb[:])
                nc.vector.tensor_copy(
                    ssb[:, HALF * h : HALF * (h + 1)], psum_s[:]
                )
            nc.scalar.dma_start(out=outs_d[:], in_=ssb[:])

    nc.compile()
    return nc


def get_nc():
    if "nc" not in _NC_CACHE:
        _NC_CACHE["nc"] = _build()
    return _NC_CACHE["nc"]


def make_in_maps(x, low, high, W):
    xt = x.T  # (IN, B) view
    au = (high - low).astype(np.float32)
    in_maps = []
    for c in range(NCORES):
        q, r = divmod(c, I_SHARDS)  # q: o-shard, r: i-shard
        isl = slice(IN_PC * r, IN_PC * (r + 1))
        xlh_flat = np.concatenate(
            [xt[isl], low[isl, None], high[isl, None]], axis=1
        ).astype(np.float32)  # (IN_PC, NB)
        # -> [partition, i-tile, col]
        xlh = np.ascontiguousarray(
            xlh_flat.reshape(IT, 128, NB).transpose(1, 0, 2).astype(WNP)
        )
        absu = np.ascontiguousarray(au[isl].reshape(IT, 128).T)
        # W block (OPC, IN_PC) -> [phase, partition, i-tile, o-col]:
        # wt[h, p, t, f] = W[OPC*q + HALF*h + f, IN_PC*r + 128*t + p]
        wblk = W[OPC * q : OPC * (q + 1), isl]
        wt = np.ascontiguousarray(
            wblk.reshape(NPH, HALF, IT, 128).transpose(0, 3, 2, 1).astype(WNP)
        )
        in_maps.append({"wt": wt, "xlh": xlh, "absu": absu})
    return in_maps


def assemble(results, b):
    yblocks = []
    sblocks = []
    for q in range(O_SHARDS):
        yb = results[q * I_SHARDS]["out_y"]
        sb = results[q * I_SHARDS]["out_s"][0]
        for r in range(1, I_SHARDS):
            yb = yb + results[q * I_SHARDS + r]["out_y"]
            sb = sb + results[q * I_SHARDS + r]["out_s"][0]
        yblocks.append(yb)
        sblocks.append(sb)
    ycols = np.concatenate(yblocks, axis=0)  # (8192, 258)
    s = np.concatenate(sblocks)  # (8192,)
    y = ycols[:, :B].T + b[None, :]
    t_low = ycols[:, B]
    t_high = ycols[:, B + 1]
    low_out = t_high - s + b
    high_out = t_low + s + b
    return (
        np.ascontiguousarray(y, dtype=np.float32),
        low_out.astype(np.float32),
        high_out.astype(np.float32),
    )


def kernel(x, low, high, W, b):
    x = np.asarray(x, dtype=np.float32)
    low = np.asarray(low, dtype=np.float32)
    high = np.asarray(high, dtype=np.float32)
    W = np.asarray(W, dtype=np.float32)
    b = np.asarray(b, dtype=np.float32)

    nc = get_nc()
    in_maps = make_in_maps(x, low, high, W)
    r = run_bass_kernel_spmd(nc, in_maps, list(range(NCORES)))
    return assemble(r.results, b)


# revision 32
# speedup vs baseline: 1.1712x; 1.0023x over previous
"""Trainium2 Bass kernel for nn_AbstractLinear (IBP linear layer).

Computes, for x (256, 8192), W (8192, 8192), b, low, high (8192,):
    y        = x @ W.T + b
    low_out  = W+ @ low + W- @ high + b
    high_out = W+ @ high + W- @ low + b

Sharding: 2D grid over the 8 cores — W split O_SHARDS-wise along output rows
and I_SHARDS-wise along the contraction dim. Each core handles a
(OPC x IN_PC) block of W; host sums the I_SHARDS partials per output block
and concatenates (no on-device collectives).

Math used on-device (per core, o = its OPC output rows, i = its IN_PC slice):
    out_y[o, 0:256] = sum_i wt[i,o] * x.T[i, :]      (partial y.T block)
    out_y[o, 256]   = partial (W @ low)[o]           (extra moving column)
    out_y[o, 257]   = partial (W @ high)[o]          (extra moving column)
    out_s[o]        = partial sum_i relu(W[o,i]) * (high-low)[i]
Host assembly (after summing i-shard partials):
    y        = out_y[:, :256].T + b
    low_out  = out_y[:, 257] - out_s + b
    high_out = out_y[:, 256] + out_s + b

W and x travel in fp16 (same 11-bit-mantissa class as the PE's fp32r/TF32
matmul path, but half the HBM bytes and FWL-accelerated weight loads);
measured rel err ~2e-4 on y, ~4e-5 on the bounds. The relu matvec is
computed as elementwise relu(wt * (high-low)) on ACT/DVE (alternating
i-tiles), 4 tiles tree-summed on DVE in fp16, then reduced over partitions
with a ones-vector fp16 matmul accumulating in PSUM.
"""

import numpy as np

import concourse.bass as bass
import concourse.tile as tile
from concourse import bacc, mybir
from concourse.bass_utils import run_bass_kernel_spmd

FP32R = mybir.dt.float32r
FP32 = mybir.dt.float32
FP16 = mybir.dt.float16
BF16 = mybir.dt.bfloat16

# W and x travel in fp16: same 11-bit mantissa class as the fp32r (TF32)
# matmul path, but half the HBM bytes, FWL-accelerated weight loads and
# 1 cycle/row unconditionally.
WDT = FP16
WNP = __import__("numpy").float16

IN = 8192  # total contraction dim i
OUT = 8192  # total output features
B = 256  # batch
NCORES = 8
O_SHARDS = 8
I_SHARDS = NCORES // O_SHARDS
OPC = OUT // O_SHARDS  # output features per core
IN_PC = IN // I_SHARDS  # contraction per core
NB = B + 2  # moving columns: 256 batch + low + high
IT = IN_PC // 128  # i-tiles per core
HALF = 512  # o columns processed per phase
NPH = OPC // HALF  # phases
OSUB = HALF // 128  # 4 psum tiles per phase
BLKI = 8  # i-tiles per steady-state wt DMA (1 MiB fp16 transfers)


def _block_schedule():
    # Ladder up: small leading transfers so the PE starts ~10 us sooner, then
    # 2 MiB steady-state blocks, then taper down so the final relu/add/s-chain
    # isn't bunched behind one big block at the end. Sizes stay multiples of 4
    # (after the first two) so s-quads never straddle a block.
    head = [1, 1, 2, 4, 8]
    tail = [8, 4, 4]
    blocks = []
    t = 0
    for n in head:
        if t + n <= IT - sum(tail):
            blocks.append((t, n))
            t += n
    while t < IT - sum(tail):
        n = min(BLKI, IT - sum(tail) - t)
        blocks.append((t, n))
        t += n
    for n in tail:
        if t + n <= IT:
            blocks.append((t, n))
            t += n
    while t < IT:
        blocks.append((t, min(4, IT - t)))
        t += min(4, IT - t)
    return blocks


BLOCKS = _block_schedule()

_NC_CACHE = {}


def _build():
    nc = bacc.Bacc("TRN2", target_bir_lowering=False, debug=False, num_devices=NCORES)
    # wt is host-pre-tiled to [phase, partition, i-tile, o-col] so every DMA
    # reads per-partition-contiguous 2 KiB * nt runs (16 KiB descriptors at
    # steady state) instead of scattered 2 KiB chunks.
    wt_d = nc.dram_tensor("wt", [NPH, 128, IT, HALF], WDT, kind="ExternalInput")
    # xlh likewise pre-tiled to [partition, i-tile, col].
    xlh_d = nc.dram_tensor("xlh", [128, IT, NB], WDT, kind="ExternalInput")
    absu_d = nc.dram_tensor("absu", [128, IT], FP32, kind="ExternalInput")
    outy_d = nc.dram_tensor("out_y", [OPC, NB], FP32, kind="ExternalOutput")
    outs_d = nc.dram_tensor("out_s", [1, OPC], FP32, kind="ExternalOutput")

    with tile.TileContext(nc) as tc:
        with (
            tc.tile_pool(name="const", bufs=1) as constp,
            tc.tile_pool(name="data", bufs=1) as datap,
            tc.tile_pool(name="wtp", bufs=10) as wtp,
            tc.tile_pool(name="tmpp", bufs=3) as tmpp,
            tc.tile_pool(name="tsump", bufs=5) as tsump,
            tc.tile_pool(name="outp", bufs=3) as outp,
            # py0..py2 double-buffered so the next phase's matmuls start while
            # the previous phase's psum drains; py3+ps single (8 banks total).
            tc.tile_pool(name="psumA", bufs=2, space="PSUM") as psumpa,
            tc.tile_pool(name="psumB", bufs=1, space="PSUM") as psumpb,
        ):
            ones = constp.tile([128, 1], FP16)
            nc.vector.memset(ones[:], 1.0)
            zbias = constp.tile([128, 1], FP32)
            nc.vector.memset(zbias[:], 0.0)

            # xlh/absu/outputs go through the ACT HWDGE ring; wt tiles through
            # the SP ring — otherwise the xlh load FIFO-blocks the first
            # weight tile and the PE idles for its duration.
            XLH = datap.tile([128, IT, NB], WDT)
            AU = datap.tile([128, IT], FP32)
            for k, (t0, nt) in enumerate(BLOCKS):
                nc.scalar.dma_start(
                    out=XLH[:, t0 : t0 + nt, :],
                    in_=xlh_d[:, t0 : t0 + nt, :],
                )
                if k == 0:
                    nc.scalar.dma_start(out=AU[:], in_=absu_d[:])

            ssb = outp.tile([1, OPC], FP32, tag="ssb")

            for h in range(NPH):
                psum_y = [
                    (psumpa if j < 3 else psumpb).tile(
                        [128, NB], FP32, name=f"py{h}_{j}", tag=f"py{j}"
                    )
                    for j in range(OSUB)
                ]
                psum_s = psumpb.tile([1, HALF], FP32, name=f"ps{h}", tag="ps")
                pend_s = []
                first_s = [True]

                w4_of = {}
                for kb, (t0, nt) in enumerate(BLOCKS):
                    w4 = wtp.tile([128, nt, HALF], WDT, tag="w4")
                    nc.sync.dma_start(
                        out=w4[:],
                        in_=wt_d[h, :, t0 : t0 + nt, :],
                    )
                    for j in range(nt):
                        t = t0 + j
                        w4_of[t] = (w4, j)
                        if t == 2:
                            # Catch up the deferred osub3 matmuls (t=0 has
                            # start=True, so it must precede t=2's osub3).
                            for td in (0, 1):
                                wb, jb = w4_of[td]
                                nc.tensor.matmul(
                                    psum_y[OSUB - 1][:],
                                    wb[:, jb, 128 * (OSUB - 1) : 128 * OSUB],
                                    XLH[:, td, :],
                                    start=(td == 0),
                                    stop=False,
                                )
                        for osub in range(OSUB):
                            # Defer the first two i-tiles' osub3 matmuls: py3
                            # is single-buffered, so give its drain from the
                            # previous phase ~1 us of PE cover.
                            if t < 2 and osub == OSUB - 1:
                                continue
                            nc.tensor.matmul(
                                psum_y[osub][:],
                                w4[:, j, 128 * osub : 128 * (osub + 1)],
                                XLH[:, t, :],
                                start=(t == 0) and osub != OSUB - 1,
                                stop=(t == IT - 1),
                            )
                        # relu(wt * (high-low)): even i-tiles on ACT, odd on
                        # DVE; 8 tiles tree-summed on DVE so only one
                        # PSUM-reducing matmul per eight i-tiles.
                        if t % 2 == 0:
                            tmpe = tmpp.tile([128, HALF], FP16, tag="tmpe")
                            nc.scalar.activation(
                                tmpe[:],
                                w4[:, j, :],
                                mybir.ActivationFunctionType.Relu,
                                bias=zbias[:],
                                scale=AU[:, t : t + 1],
                            )
                        else:
                            tmpo = tmpp.tile([128, HALF], FP16, tag="tmpo")
                            nc.vector.tensor_scalar(
                                tmpo[:],
                                w4[:, j, :],
                                0.0,
                                AU[:, t : t + 1],
                                mybir.AluOpType.max,
                                mybir.AluOpType.mult,
                            )
                            if t % 4 == 1:
                                tsumA = tmpp.tile([128, HALF], FP16, tag="tsumA")
                                nc.vector.tensor_add(tsumA[:], tmpe[:], tmpo[:])
                            elif t % 8 == 3:
                                tsumQ = tmpp.tile([128, HALF], FP16, tag="tsumQ")
                                nc.vector.tensor_add(tsumQ[:], tmpe[:], tmpo[:])
                                tsumO = tsump.tile([128, HALF], FP16, tag="tsumO")
                                nc.vector.tensor_add(tsumO[:], tsumA[:], tsumQ[:])
                            else:
                                tsumB = tmpp.tile([128, HALF], FP16, tag="tsumB")
                                nc.vector.tensor_add(tsumB[:], tmpe[:], tmpo[:])
                                tsumC = tmpp.tile([128, HALF], FP16, tag="tsumC")
                                nc.vector.tensor_add(tsumC[:], tsumA[:], tsumB[:])
                                tsum = tsump.tile([128, HALF], FP16, tag="tsum")
                                nc.vector.tensor_add(tsum[:], tsumO[:], tsumC[:])
                                # psum_s is single-buffered and its tsum
                                # operand comes off the (laggy) DVE chain:
                                # roll a 1-oct deferral so the in-order PE
                                # never waits on either, flushing at phase end.
                                pend_s.append(tsum)
                                keep = 1 if t < IT - 1 else 0
                                while len(pend_s) > keep:
                                    ts2 = pend_s.pop(0)
                                    nc.tensor.matmul(
                                        psum_s[:],
                                        ones[:],
                                        ts2[:],
                                        start=first_s[0],
                                        stop=(t == IT - 1 and not pend_s),
                                    )
                                    first_s[0] = False

                # drain py3 first: the next phase's deferred osub3 matmuls
                # are what ultimately wait on this bank.
                for osub in (3, 0, 1, 2):
                    ysb = outp.tile([128, NB], FP32, tag="ysb")
                    nc.vector.tensor_copy(ysb[:], psum_y[osub][:])
                    o0 = HALF * h + 128 * osub
                    nc.scalar.dma_start(out=outy_d[o0 : o0 + 128, :], in_=ysb[:])
                nc.vector.tensor_copy(
                    ssb[:, HALF * h : HALF * (h + 1)], psum_s[:]
                )
            nc.scalar.dma_start(out=outs_d[:], in_=ssb[:])

    nc.compile()
    return nc


def get_nc():
    if "nc" not in _NC_CACHE:
        _NC_CACHE["nc"] = _build()
    return _NC_CACHE["nc"]


def make_in_maps(x, low, high, W):
    xt = x.T  # (IN, B) view
    au = (high - low).astype(np.float32)
    in_maps = []
    for c in range(NCORES):
        q, r = divmod(c, I_SHARDS)  # q: o-shard, r: i-shard
        isl = slice(IN_PC * r, IN_PC * (r + 1))
        xlh_flat = np.concatenate(
            [xt[isl], low[isl, None], high[isl, None]], axis=1
        ).astype(np.float32)  # (IN_PC, NB)
        # -> [partition, i-tile, col]
        xlh = np.ascontiguousarray(
            xlh_flat.reshape(IT, 128, NB).transpose(1, 0, 2).astype(WNP)
        )
        absu = np.ascontiguousarray(au[isl].reshape(IT, 128).T)
        # W block (OPC, IN_PC) -> [phase, partition, i-tile, o-col]:
        # wt[h, p, t, f] = W[OPC*q + HALF*h + f, IN_PC*r + 128*t + p]
        wblk = W[OPC * q : OPC * (q + 1), isl]
        wt = np.ascontiguousarray(
            wblk.reshape(NPH, HALF, IT, 128).transpose(0, 3, 2, 1).astype(WNP)
        )
        in_maps.append({"wt": wt, "xlh": xlh, "absu": absu})
    return in_maps


def assemble(results, b):
    yblocks = []
    sblocks = []
    for q in range(O_SHARDS):
        yb = results[q * I_SHARDS]["out_y"]
        sb = results[q * I_SHARDS]["out_s"][0]
        for r in range(1, I_SHARDS):
            yb = yb + results[q * I_SHARDS + r]["out_y"]
            sb = sb + results[q * I_SHARDS + r]["out_s"][0]
        yblocks.append(yb)
        sblocks.append(sb)
    ycols = np.concatenate(yblocks, axis=0)  # (8192, 258)
    s = np.concatenate(sblocks)  # (8192,)
    y = ycols[:, :B].T + b[None, :]
    t_low = ycols[:, B]
    t_high = ycols[:, B + 1]
    low_out = t_high - s + b
    high_out = t_low + s + b
    return (
        np.ascontiguousarray(y, dtype=np.float32),
        low_out.astype(np.float32),
        high_out.astype(np.float32),
    )


def kernel(x, low, high, W, b):
    x = np.asarray(x, dtype=np.float32)
    low = np.asarray(low, dtype=np.float32)
    high = np.asarray(high, dtype=np.float32)
    W = np.asarray(W, dtype=np.float32)
    b = np.asarray(b, dtype=np.float32)

    nc = get_nc()
    in_maps = make_in_maps(x, low, high, W)
    r = run_bass_kernel_spmd(nc, in_maps, list(range(NCORES)))
    return assemble(r.results, b)
